# revision 12
# baseline (speedup 1.0000x reference)
"""CAM (channel attention module) Trainium2 Bass kernel — bf16-resident v2.

Reference computation (per sample, x: [C, N] with N = H*W):
    energy    = x @ x.T                      # [C, C] Gram matrix
    att       = softmax(rowmax(energy) - energy, axis=-1)
              = softmax(-energy, axis=-1)    # identical after max-shift
    out       = att @ x                      # [C, N]
    result    = gamma * out + x
Sharding: data-parallel over batch, B=16 -> 2 samples per core on 8 cores.

v2 design (HBM floor: 16 MiB in + 16 MiB out per sample, ~187 us/core):
  - x is DMA'd f32->bf16 in flight (SWDGE cast on nc.gpsimd) in 1 MiB
    loads and kept resident in SBUF as bf16: 8 MiB/sample, so BOTH
    samples prefetch fully (16 MiB) and the DMA never starves while a
    sample's softmax serializes phase 1 -> phase 2 (the v1 f32-resident
    kernel lost ~22 us to exactly that on its two phase boundaries).
  - Phase 1: PE transposes bf16 tiles (ACT/DVE-alternated PSUM evict),
    bf16 energy matmuls accumulate f32 in PSUM.
  - Softmax row-min shift, exp on ACT (bf16 att + f32 row-sum), 1/denom
    folded into the phase-2 eviction scale.
  - Phase 2: att^T stationary, resident bf16 x as the moving operand
    directly (native dtype - the fp32r rounding copies of v1 are gone),
    eviction osb = gamma/denom * psum + x on DVE/Pool alternated, f32
    stores batched 2 chunks (512 KiB) wide.
"""

import threading

import numpy as np

import concourse.bass as bass
import concourse.mybir as mybir
import concourse.tile as tile
from concourse import bacc
from concourse.bass_utils import run_bass_kernel_spmd
from concourse.masks import make_identity

P = 128
F32 = mybir.dt.float32
BF16 = mybir.dt.bfloat16

# Full-problem shapes (hardcoded per harness contract).
B_FULL = 16
C_FULL = 256
H_FULL = W_FULL = 128
N_CORES = 8
B_PER_CORE = B_FULL // N_CORES  # 2


def emit_cam(tc, x, gamma_b, out, n_s, C, N, lw=2048, chunk=512, sw=512,
             xft_bufs=6, osb_bufs=16, ptr_bufs=3, pout_bufs=3,
             evict_engines=("scalar",),
             stt_engines=("vector",),
             tpc=None, tpc_sched="8x8,4x8,2x16", interleave=True,
             p1_first=True, tr_batch=4):
    """Emit the per-core CAM kernel (bf16-resident).

    x:       DRAM [n_s, C, N] f32
    gamma_b: DRAM [128, 1] f32 (gamma broadcast to all partitions on host)
    out:     DRAM [n_s, C, N] f32
    lw:      load width (columns per cast-DMA; 2048 = 1 MiB f32 payload)
    chunk:   phase-2 matmul chunk width (<=512: one PSUM bank f32)
    sw:      store width (columns per output DMA, multiple of chunk)
    """
    nc = tc.nc
    cb_n = C // P            # channel blocks (2)
    nt = N // P              # phase-1 transpose steps per sample (128)
    nld = N // lw            # loads per channel block per sample (8)
    nch = N // chunk         # phase-2 chunks (32)
    spc = sw // chunk        # chunks per store tile (2)
    if tpc is None:
        tpc = nt // nch      # p1 steps interleaved per p2 chunk (4)
    assert C == 2 * P and lw % P == 0 and sw % chunk == 0 and chunk <= 512

    eng = {"scalar": nc.scalar, "vector": nc.vector, "gpsimd": nc.gpsimd}

    with (
        tc.tile_pool(name="consts", bufs=1) as consts,
        tc.tile_pool(name="xf", bufs=n_s * cb_n * nld) as xf_pool,
        tc.tile_pool(name="xft", bufs=xft_bufs) as xft_pool,
        tc.tile_pool(name="att", bufs=2 * cb_n) as att_pool,
        tc.tile_pool(name="attT", bufs=2 * cb_n) as attT_pool,
        tc.tile_pool(name="osb", bufs=osb_bufs) as osb_pool,
        tc.tile_pool(name="stat", bufs=4) as stat_pool,
        tc.tile_pool(name="eps", bufs=2, space="PSUM") as eps_pool,
        tc.tile_pool(name="ptr", bufs=ptr_bufs, space="PSUM") as ptr_pool,
        tc.tile_pool(name="pout", bufs=pout_bufs, space="PSUM") as pout_pool,
    ):
        ident = consts.tile([P, P], BF16, tag="identity")
        make_identity(nc, ident)
        gamma_sb = consts.tile([P, 1], F32, tag="gamma")

        # -------- per-sample stage emitters (state dict per sample) ------
        def new_state(s):
            return {"s": s, "xf": [[None] * nld for _ in range(cb_n)],
                    "e_ps": None, "prev": None, "attT": None, "ginv": None,
                    "osb": [None] * cb_n}

        def emit_load(st, o):
            # SWDGE cast DMA: f32 HBM payload, bf16 SBUF destination.
            s = st["s"]
            for cb in range(cb_n):
                t_ = xf_pool.tile([P, lw], BF16, tag="xf",
                                  name=f"xf_s{s}_c{cb}_o{o}")
                nc.gpsimd.dma_start(
                    t_, x[s, cb * P:(cb + 1) * P, o * lw:(o + 1) * lw])
                st["xf"][cb][o] = t_

        def emit_tr_group(st, tg, nt_g):
            # transpose nt_g consecutive t-steps into ONE PSUM tile and
            # evict with a single wide copy (amortizes the per-op cost).
            # Column range [k*C, (k+1)*C) of the group tile holds step
            # tg+k's [128n, 256c] block; each consumer matmul only reads
            # its own range, so mixing n-blocks across ranges is fine.
            s = st["s"]
            ptr = ptr_pool.tile([P, nt_g * C], BF16, tag="ptr",
                                name=f"ptr_s{s}_g{tg}")
            for k in range(nt_g):
                o, lc = divmod((tg + k) * P, lw)
                for cb in range(cb_n):
                    nc.tensor.transpose(
                        ptr[:, k * C + cb * P:k * C + (cb + 1) * P],
                        st["xf"][cb][o][:, lc:lc + P], ident)
            xft = xft_pool.tile([P, nt_g * C], BF16, tag="xft",
                                name=f"xft_s{s}_g{tg}")
            ev = eng[evict_engines[tg % len(evict_engines)]]
            if ev is nc.scalar:
                ev.copy(xft, ptr)
            else:
                ev.tensor_copy(xft, ptr)
            return xft

        def emit_mm(st, t, xft, k):
            for mb in range(cb_n):
                nc.tensor.matmul(
                    st["e_ps"][mb],
                    lhsT=xft[:, k * C + mb * P:k * C + (mb + 1) * P],
                    rhs=xft[:, k * C:(k + 1) * C],
                    start=(t == 0), stop=(t == nt - 1))

        def p1_step(st, t):
            # software-pipelined per GROUP of tr_batch steps: transposes +
            # one wide eviction for group j are emitted before group j-1's
            # accumulating matmuls, so the PE queue never head-of-line
            # blocks on the eviction copy
            if st["e_ps"] is None:
                s = st["s"]
                st["e_ps"] = [eps_pool.tile([P, C], F32, tag="eps",
                                            name=f"eps_s{s}_m{mb}")
                              for mb in range(cb_n)]
            if t % tr_batch:
                return
            nt_g = min(tr_batch, nt - t)
            xft = emit_tr_group(st, t, nt_g)
            if st["prev"] is not None:
                pt, pxft, png = st["prev"]
                for k in range(png):
                    emit_mm(st, pt + k, pxft, k)
            st["prev"] = (t, xft, nt_g)

        def p1_flush(st):
            if st["prev"] is not None:
                pt, pxft, png = st["prev"]
                for k in range(png):
                    emit_mm(st, pt + k, pxft, k)
                st["prev"] = None

        def emit_softmax(st):
            # att = exp(rowmin - energy) in bf16; denom = f32 rowsum;
            # attT tiles (stationary operand of phase 2)
            s = st["s"]
            att = []
            ginv = []
            for mb in range(cb_n):
                a = att_pool.tile([P, C], BF16, tag="att",
                                  name=f"att_s{s}_{mb}")
                den = stat_pool.tile([P, 1], F32, tag="den",
                                     name=f"den_s{s}_{mb}")
                m = stat_pool.tile([P, 1], F32, tag="m", name=f"m_s{s}_{mb}")
                nc.vector.tensor_reduce(
                    m, st["e_ps"][mb], axis=mybir.AxisListType.X,
                    op=mybir.AluOpType.min)
                nc.scalar.activation(
                    a, st["e_ps"][mb], mybir.ActivationFunctionType.Exp,
                    bias=m, scale=-1.0, accum_out=den)
                inv = stat_pool.tile([P, 1], F32, tag="inv",
                                     name=f"inv_s{s}_{mb}")
                nc.vector.reciprocal(inv, den)
                gi = stat_pool.tile([P, 1], F32, tag="gi", name=f"gi_s{s}_{mb}")
                nc.vector.tensor_tensor(gi, inv, gamma_sb,
                                        mybir.AluOpType.mult)
                att.append(a)
                ginv.append(gi)
            attT = []
            for jb in range(cb_n):
                ptr2 = ptr_pool.tile([P, C], BF16, tag="ptr",
                                     name=f"ptrT_s{s}_{jb}")
                for ib in range(cb_n):
                    nc.tensor.transpose(
                        ptr2[:, ib * P:(ib + 1) * P],
                        att[ib][:, jb * P:(jb + 1) * P], ident)
                aT = attT_pool.tile([P, C], BF16, tag="attT",
                                    name=f"attT_s{s}_{jb}")
                nc.scalar.copy(aT, ptr2)
                attT.append(aT)
            st["attT"] = attT
            st["ginv"] = ginv

        def p2_chunk(st, ch):
            # out = gamma/denom * (att @ x) + x for one chunk; the moving
            # operand is the resident bf16 x directly
            s = st["s"]
            o, lc = divmod(ch * chunk, lw)
            sg, sc = divmod(ch, spc)    # store group / chunk-in-store
            for cb in range(cb_n):
                if sc == 0:
                    st["osb"][cb] = osb_pool.tile(
                        [P, sw], F32, tag="osb", name=f"osb_s{s}_g{sg}_{cb}")
                po = pout_pool.tile([P, chunk], F32, tag="pout",
                                    name=f"po_s{s}_c{ch}_{cb}")
                for jb in range(cb_n):
                    nc.tensor.matmul(
                        po,
                        lhsT=st["attT"][jb][:, cb * P:(cb + 1) * P],
                        rhs=st["xf"][jb][o][:, lc:lc + chunk],
                        start=(jb == 0), stop=(jb == cb_n - 1))
                ev = eng[stt_engines[(ch * cb_n + cb) % len(stt_engines)]]
                ev.scalar_tensor_tensor(
                    st["osb"][cb][:, sc * chunk:(sc + 1) * chunk],
                    po, st["ginv"][cb], st["xf"][cb][o][:, lc:lc + chunk],
                    op0=mybir.AluOpType.mult, op1=mybir.AluOpType.add)
                if sc == spc - 1:
                    nc.sync.dma_start(
                        out[s, cb * P:(cb + 1) * P, sg * sw:(sg + 1) * sw],
                        st["osb"][cb])

        # -------- schedule --------
        # Both samples' loads are issued up front (everything fits in SBUF
        # as bf16), so the DMA engines always have load traffic to drain
        # while a sample's phase-1/softmax latency runs. Sample s+1's
        # phase-1 steps interleave with sample s's phase-2 chunks.
        states = [new_state(s) for s in range(n_s)]
        for st in states:
            for o in range(nld):
                emit_load(st, o)
            if st["s"] == 0:
                # tiny gamma DMA issued behind the first sample's loads so
                # it never delays the bulk stream at kernel start
                nc.sync.dma_start(gamma_sb, gamma_b)
        st0 = states[0]
        for t in range(nt):
            p1_step(st0, t)
        p1_flush(st0)
        emit_softmax(st0)
        for s in range(n_s):
            st = states[s]
            nxt_st = states[s + 1] if s + 1 < n_s else None
            if interleave and nxt_st is not None and nt % nch == 0:
                # tpc_sched: "6x16,2x16" -> per-chunk p1-step counts
                if tpc_sched:
                    counts = []
                    for part in tpc_sched.split(","):
                        c_, n_ = part.split("x")
                        counts += [int(c_)] * int(n_)
                    assert len(counts) == nch and sum(counts) == nt, \
                        (counts, nch, nt)
                else:
                    counts = [tpc] * nch
                tnext = 0
                for ch in range(nch):
                    if p1_first:
                        for _ in range(counts[ch]):
                            p1_step(nxt_st, tnext)
                            tnext += 1
                        p2_chunk(st, ch)
                    else:
                        p2_chunk(st, ch)
                        for _ in range(counts[ch]):
                            p1_step(nxt_st, tnext)
                            tnext += 1
                p1_flush(nxt_st)
                emit_softmax(nxt_st)
            else:
                for ch in range(nch):
                    p2_chunk(st, ch)
                if nxt_st is not None:
                    for t in range(nt):
                        p1_step(nxt_st, t)
                    p1_flush(nxt_st)
                    emit_softmax(nxt_st)


def build_nc(n_s=B_PER_CORE, C=C_FULL, N=H_FULL * W_FULL, **kwargs):
    nc = bacc.Bacc("TRN2", target_bir_lowering=False, debug=False)
    x = nc.dram_tensor("x", [n_s, C, N], F32, kind="ExternalInput").ap()
    gamma_b = nc.dram_tensor("gamma_b", [P, 1], F32, kind="ExternalInput").ap()
    out = nc.dram_tensor("out", [n_s, C, N], F32, kind="ExternalOutput").ap()
    with tile.TileContext(nc) as tc:
        emit_cam(tc, x, gamma_b, out, n_s, C, N, **kwargs)
    nc.compile()
    return nc


_CACHE = threading.Lock()
_NC = None


def _get_nc():
    global _NC
    with _CACHE:
        if _NC is None:
            _NC = build_nc()
    return _NC


def run_spmd(x, gamma, **kwargs):
    """Shard inputs over 8 cores, run, gather. Returns (output, results)."""
    x = np.ascontiguousarray(np.asarray(x), dtype=np.float32)
    assert x.shape == (B_FULL, C_FULL, H_FULL, W_FULL), x.shape
    n = H_FULL * W_FULL
    xs = x.reshape(B_FULL, C_FULL, n)
    gb = np.full((P, 1), np.float32(np.asarray(gamma)), dtype=np.float32)
    in_maps = [
        {"x": xs[c * B_PER_CORE:(c + 1) * B_PER_CORE], "gamma_b": gb}
        for c in range(N_CORES)
    ]
    nc = _get_nc()
    res = run_bass_kernel_spmd(nc, in_maps, core_ids=list(range(N_CORES)),
                               **kwargs)
    outs = np.stack([res.results[c]["out"] for c in range(N_CORES)])
    full = outs.reshape(B_FULL, C_FULL, H_FULL, W_FULL).astype(np.float32,
                                                               copy=False)
    return full, res


def kernel(x, gamma):
    out, _ = run_spmd(x, gamma)
    return out


# revision 15
# speedup vs baseline: 1.5721x; 1.5721x over previous
"""CAM (channel attention module) Trainium2 Bass kernel.

Reference computation (per sample, x: [C, N] with N = H*W):
    energy    = x @ x.T                      # [C, C] Gram matrix
    att       = softmax(rowmax(energy) - energy, axis=-1)
              = softmax(-energy, axis=-1)    # identical after max-shift
    out       = att @ x                      # [C, N]
    result    = gamma * out + x

Sharding: data-parallel over batch, B=16 -> 2 samples per core on 8 cores.

Per-core dataflow (per sample):
  - x [256, 16384] f32 stays resident in SBUF (16 MiB) as 2x32 tiles of
    [128, 512], loaded once from HBM; extra pool slots let the next
    sample's loads prefetch while this one computes.
  - Phase 1: PE transposes build [128n, 256c] tiles of x^T on the fly
    (ScalarE evicts them from PSUM with an fp32->fp32r rounding copy);
    two accumulating fp32r matmuls per n-tile produce energy in PSUM.
  - Softmax: row-min shift (equivalent to the reference's max-shifted
    softmax), exp on ScalarE with fused row-sum; the 1/denom
    normalization is folded into the phase-2 PSUM eviction scale.
  - Phase 2: out = E^T.T @ x with E^T (transposed unnormalized exp
    matrix) stationary and GpSimd producing the fp32r-rounded moving
    operand; eviction computes gamma/denom * psum + x in one VectorE op
    (keeping the +x residual bit-exact) and streams to HBM.

HBM traffic is the floor: 16 MiB in + 16 MiB out per sample; the
cost-model timeline puts one full invocation at ~224 us per core against
a ~186 us DMA roofline.
"""

import threading

import numpy as np

import concourse.bass as bass
import concourse.mybir as mybir
import concourse.tile as tile
from concourse import bacc
from concourse.bass_utils import run_bass_kernel_spmd
from concourse.masks import make_identity

P = 128
F32 = mybir.dt.float32
F32R = mybir.dt.float32r

# Full-problem shapes (hardcoded per harness contract).
B_FULL = 16
C_FULL = 256
H_FULL = W_FULL = 128
N_CORES = 8
B_PER_CORE = B_FULL // N_CORES  # 2


def emit_cam(tc, x, gamma_b, out, n_s, C, N, xt_cols=512, chunk=512,
             xf_bufs=None, osb_bufs=6, xfr_bufs=6, ptr_bufs=3, pout_bufs=3,
             xft_bufs=6, xfr_engine="gpsimd", tr_f32r=False, xft_split=False,
             evict_act_every=0, interleave=True, p1_first=False, sym=False,
             store_engine="sync", load_engine="sync"):
    """Emit the per-core CAM kernel.

    x:       DRAM [n_s, C, N] f32
    gamma_b: DRAM [128, 1] f32 (gamma broadcast to all partitions on host)
    out:     DRAM [n_s, C, N] f32
    """
    nc = tc.nc
    cb_n = C // P            # channel blocks (2)
    nt = N // P              # n-tiles for transposes
    nxt = N // xt_cols       # resident xf tiles per channel block
    nch = N // chunk         # phase-2 output chunks
    assert xt_cols % P == 0 and xt_cols % chunk == 0 and C == 256

    if xf_bufs is None:
        xf_bufs = 2 * nxt + 18
    xfr_copy = nc.gpsimd.tensor_copy if xfr_engine == "gpsimd" \
        else nc.vector.tensor_copy
    with (
        tc.tile_pool(name="consts", bufs=1) as consts,
        tc.tile_pool(name="xf", bufs=xf_bufs) as xf_pool,
        tc.tile_pool(name="xft", bufs=xft_bufs) as xft_pool,
        tc.tile_pool(name="att", bufs=4) as att_pool,
        tc.tile_pool(name="attT", bufs=4) as attT_pool,
        tc.tile_pool(name="osb", bufs=osb_bufs) as osb_pool,
        tc.tile_pool(name="xfr", bufs=xfr_bufs) as xfr_pool,
        tc.tile_pool(name="stat", bufs=4) as stat_pool,
        tc.tile_pool(name="eps", bufs=2, space="PSUM") as eps_pool,
        tc.tile_pool(name="ptr", bufs=ptr_bufs, space="PSUM") as ptr_pool,
        tc.tile_pool(name="pout", bufs=pout_bufs, space="PSUM") as pout_pool,
    ):
        identity = consts.tile([P, P], F32, tag="identity")
        make_identity(nc, identity)
        identity_r = identity
        if tr_f32r:
            identity_r = consts.tile([P, P], F32R, tag="identity_r")
            make_identity(nc, identity_r)
        gamma_sb = consts.tile([P, 1], F32, tag="gamma")
        nc.sync.dma_start(gamma_sb, gamma_b)

        # -------- per-sample stage emitters (state dict per sample) --------
        def new_state(s):
            return {"s": s, "xf": [[None] * nxt for _ in range(cb_n)],
                    "e_ps": None, "prev": None, "attT": None, "ginv": None}

        def emit_load(st, o):
            # Tiles stay f32: a float32r-typed DMA destination makes the DGE
            # round the payload to fp32r precision in flight, which would
            # corrupt the exact residual copy of x.
            s = st["s"]
            for cb in range(cb_n):
                t_ = xf_pool.tile([P, xt_cols], F32, tag="xf",
                                  name=f"xf_s{s}_c{cb}_o{o}")
                getattr(nc, load_engine).dma_start(
                    t_, x[s, cb * P:(cb + 1) * P, o * xt_cols:(o + 1) * xt_cols])
                st["xf"][cb][o] = t_

        def emit_tr(st, t):
            s = st["s"]
            o, lc = divmod(t * P, xt_cols)
            ptr = ptr_pool.tile([P, C], F32R if tr_f32r else F32,
                                tag="ptr", name=f"ptr_s{s}_t{t}")
            for cb in range(cb_n):
                src = st["xf"][cb][o][:, lc:lc + P]
                if tr_f32r:
                    src = src.bitcast(F32R)
                nc.tensor.transpose(
                    ptr[:, cb * P:(cb + 1) * P], src, identity_r)
            xft = xft_pool.tile([P, C], F32R, tag="xft", name=f"xft_s{s}_t{t}")
            if xft_split and t % 2 == 0:
                nc.vector.tensor_copy(xft, ptr)
            else:
                nc.scalar.copy(xft, ptr)
            return xft

        def emit_mm(st, t, xft):
            # sym=True computes only energy[128:256, 128:256] for mb=1 and
            # rebuilds the lower-left block by transpose. It LOSES in the
            # cost model (248 vs 220 us): a 128-wide fp32r moving operand
            # falls off the >=256 fast path to 4 cycles/row. Kept for
            # documentation; default off.
            for mb in range(cb_n):
                rhs = xft if not (sym and mb == 1) else xft[:, P:C]
                nc.tensor.matmul(
                    st["e_ps"][mb],
                    lhsT=xft[:, mb * P:(mb + 1) * P],
                    rhs=rhs,
                    start=(t == 0), stop=(t == nt - 1))

        def p1_step(st, t):
            # software-pipelined one step: transpose+copy for tile t are
            # emitted before the accumulating matmuls of tile t-1, so the PE
            # engine order never blocks on the ScalarE copy
            if st["e_ps"] is None:
                s = st["s"]
                st["e_ps"] = [eps_pool.tile([P, C if not (sym and mb == 1)
                                             else P], F32, tag="eps",
                                            name=f"eps_s{s}_m{mb}")
                              for mb in range(cb_n)]
            xft = emit_tr(st, t)
            if st["prev"] is not None:
                emit_mm(st, *st["prev"])
            st["prev"] = (t, xft)

        def p1_flush(st):
            if st["prev"] is not None:
                emit_mm(st, *st["prev"])
                st["prev"] = None

        def emit_softmax(st):
            # E = exp(rowmin - energy); denom = rowsum(E); then E^T tiles
            # (stationary operand of phase 2)
            s = st["s"]
            e10_ps = None
            if sym:
                # reconstruct energy[128:256, 0:128] = energy[0:128,128:256].T
                e01 = xft_pool.tile([P, P], F32, tag="xft", name=f"e01_s{s}")
                nc.scalar.copy(e01, st["e_ps"][0][:, P:C])
                e10_ps = ptr_pool.tile([P, P], F32, tag="ptr", name=f"e10_s{s}")
                nc.tensor.transpose(e10_ps, e01, identity)
            att = []
            ginv = []
            for mb in range(cb_n):
                a = att_pool.tile([P, C], F32, tag="att", name=f"att_s{s}_{mb}")
                den = stat_pool.tile([P, 1], F32, tag="den", name=f"den_s{s}_{mb}")
                if sym and mb == 1:
                    # row of block 1 lives in two PSUM pieces: [e10 | e_ps[1]]
                    ma = stat_pool.tile([P, 1], F32, tag="ma", name=f"ma_s{s}")
                    mb_ = stat_pool.tile([P, 1], F32, tag="mb", name=f"mb_s{s}")
                    nc.vector.tensor_reduce(
                        ma, e10_ps, axis=mybir.AxisListType.X,
                        op=mybir.AluOpType.min)
                    nc.vector.tensor_reduce(
                        mb_, st["e_ps"][1], axis=mybir.AxisListType.X,
                        op=mybir.AluOpType.min)
                    m = stat_pool.tile([P, 1], F32, tag="m", name=f"m_s{s}_{mb}")
                    nc.vector.tensor_tensor(m, ma, mb_, mybir.AluOpType.min)
                    db = stat_pool.tile([P, 1], F32, tag="db", name=f"db_s{s}")
                    nc.scalar.activation(
                        a[:, 0:P], e10_ps, mybir.ActivationFunctionType.Exp,
                        bias=m, scale=-1.0, accum_out=den)
                    nc.scalar.activation(
                        a[:, P:C], st["e_ps"][1],
                        mybir.ActivationFunctionType.Exp,
                        bias=m, scale=-1.0, accum_out=db)
                    nc.vector.tensor_tensor(den, den, db, mybir.AluOpType.add)
                else:
                    m = stat_pool.tile([P, 1], F32, tag="m", name=f"m_s{s}_{mb}")
                    nc.vector.tensor_reduce(
                        m, st["e_ps"][mb], axis=mybir.AxisListType.X,
                        op=mybir.AluOpType.min)
                    nc.scalar.activation(
                        a, st["e_ps"][mb], mybir.ActivationFunctionType.Exp,
                        bias=m, scale=-1.0, accum_out=den)
                inv = stat_pool.tile([P, 1], F32, tag="inv", name=f"inv_s{s}_{mb}")
                nc.vector.reciprocal(inv, den)
                gi = stat_pool.tile([P, 1], F32, tag="gi", name=f"gi_s{s}_{mb}")
                nc.vector.tensor_tensor(gi, inv, gamma_sb, mybir.AluOpType.mult)
                att.append(a)
                ginv.append(gi)
            attT = []
            for jb in range(cb_n):
                ptr2 = ptr_pool.tile([P, C], F32, tag="ptr", name=f"ptrT_s{s}_{jb}")
                for ib in range(cb_n):
                    nc.tensor.transpose(
                        ptr2[:, ib * P:(ib + 1) * P],
                        att[ib][:, jb * P:(jb + 1) * P], identity)
                aT = attT_pool.tile([P, C], F32R, tag="attT",
                                    name=f"attT_s{s}_{jb}")
                nc.scalar.copy(aT, ptr2)
                attT.append(aT)
            st["attT"] = attT
            st["ginv"] = ginv

        def p2_chunk(st, ch):
            # out = gamma/denom * (E^T.T @ xf) + xf for one 512-column chunk
            s = st["s"]
            o, lc = divmod(ch * chunk, xt_cols)
            # fp32r-rounded copy of this chunk (walrus requires fp32r matmul
            # operands to come from a rounding instruction; the rounded copy
            # is reused by both output channel blocks)
            xfr = []
            for jb in range(cb_n):
                r_ = xfr_pool.tile([P, chunk], F32R, tag="xfr",
                                   name=f"xfr_s{s}_c{ch}_{jb}")
                if s == n_s - 1 and jb == 1:
                    # last sample's phase 2 runs with ScalarE idle; split the
                    # rounding copies Pool/ACT to hit the store floor
                    nc.scalar.copy(r_, st["xf"][jb][o][:, lc:lc + chunk])
                else:
                    xfr_copy(r_, st["xf"][jb][o][:, lc:lc + chunk])
                xfr.append(r_)
            for cb in range(cb_n):
                po = pout_pool.tile([P, chunk], F32, tag="pout",
                                    name=f"po_s{s}_c{ch}_{cb}")
                for jb in range(cb_n):
                    nc.tensor.matmul(
                        po,
                        lhsT=st["attT"][jb][:, cb * P:(cb + 1) * P],
                        rhs=xfr[jb],
                        start=(jb == 0), stop=(jb == cb_n - 1))
                osb = osb_pool.tile([P, chunk], F32, tag="osb",
                                    name=f"osb_s{s}_c{ch}_{cb}")
                if evict_act_every and ch % evict_act_every == 0:
                    tmp = osb_pool.tile([P, chunk], F32, tag="etmp",
                                        name=f"etmp_s{s}_c{ch}_{cb}")
                    nc.scalar.mul(tmp, po, st["ginv"][cb])
                    nc.vector.tensor_tensor(
                        osb, tmp, st["xf"][cb][o][:, lc:lc + chunk],
                        mybir.AluOpType.add)
                else:
                    nc.vector.scalar_tensor_tensor(
                        osb, po, st["ginv"][cb], st["xf"][cb][o][:, lc:lc + chunk],
                        op0=mybir.AluOpType.mult, op1=mybir.AluOpType.add)
                getattr(nc, store_engine).dma_start(
                    out[s, cb * P:(cb + 1) * P, ch * chunk:(ch + 1) * chunk], osb)

        # -------- schedule --------
        # Sample s's phase 2 is emitted interleaved with sample s+1's loads
        # and phase-1 steps, so the next sample's pipeline keeps pace with
        # its trickling loads instead of piling up a tail backlog.
        states = [new_state(s) for s in range(n_s)]
        st0 = states[0]
        for o in range(nxt):
            emit_load(st0, o)
        for t in range(nt):
            p1_step(st0, t)
        p1_flush(st0)
        emit_softmax(st0)
        for s in range(n_s):
            st = states[s]
            nxt_st = states[s + 1] if s + 1 < n_s else None
            if interleave and nxt_st is not None and nxt % nch == 0 \
                    and nt % nch == 0:
                opc = nxt // nch
                tpc = nt // nch
                for ch in range(nch):
                    for k in range(opc):
                        emit_load(nxt_st, ch * opc + k)
                    if p1_first:
                        for k in range(tpc):
                            p1_step(nxt_st, ch * tpc + k)
                        p2_chunk(st, ch)
                    else:
                        p2_chunk(st, ch)
                        for k in range(tpc):
                            p1_step(nxt_st, ch * tpc + k)
                p1_flush(nxt_st)
                emit_softmax(nxt_st)
            else:
                for ch in range(nch):
                    p2_chunk(st, ch)
                if nxt_st is not None:
                    for o in range(nxt):
                        emit_load(nxt_st, o)
                    for t in range(nt):
                        p1_step(nxt_st, t)
                    p1_flush(nxt_st)
                    emit_softmax(nxt_st)


def build_nc(n_s=B_PER_CORE, C=C_FULL, N=H_FULL * W_FULL, **kwargs):
    nc = bacc.Bacc("TRN2", target_bir_lowering=False, debug=False)
    x = nc.dram_tensor("x", [n_s, C, N], F32, kind="ExternalInput").ap()
    gamma_b = nc.dram_tensor("gamma_b", [P, 1], F32, kind="ExternalInput").ap()
    out = nc.dram_tensor("out", [n_s, C, N], F32, kind="ExternalOutput").ap()
    with tile.TileContext(nc) as tc:
        emit_cam(tc, x, gamma_b, out, n_s, C, N, **kwargs)
    nc.compile()
    return nc


_CACHE = threading.Lock()
_NC = None


def _get_nc():
    global _NC
    with _CACHE:
        if _NC is None:
            _NC = build_nc()
    return _NC


def run_spmd(x, gamma, **kwargs):
    """Shard inputs over 8 cores, run, gather. Returns (output, BassKernelResults)."""
    x = np.ascontiguousarray(np.asarray(x), dtype=np.float32)
    assert x.shape == (B_FULL, C_FULL, H_FULL, W_FULL), x.shape
    n = H_FULL * W_FULL
    xs = x.reshape(B_FULL, C_FULL, n)
    gb = np.full((P, 1), np.float32(np.asarray(gamma)), dtype=np.float32)
    in_maps = [
        {"x": xs[c * B_PER_CORE:(c + 1) * B_PER_CORE], "gamma_b": gb}
        for c in range(N_CORES)
    ]
    nc = _get_nc()
    res = run_bass_kernel_spmd(nc, in_maps, core_ids=list(range(N_CORES)), **kwargs)
    outs = np.stack([res.results[c]["out"] for c in range(N_CORES)])
    full = outs.reshape(B_FULL, C_FULL, H_FULL, W_FULL).astype(np.float32, copy=False)
    return full, res


def kernel(x, gamma):
    out, _ = run_spmd(x, gamma)
    return out

